# revision 2
# baseline (speedup 1.0000x reference)
"""Trainium2 Bass kernel for:
    out = sigmoid(cos(pi * x[:, 0, :510, :510] + weight[0]) - threshold[0])[:, None]

x: [64, 1, 512, 512] f32, weight: [9] f32, threshold: [1] f32.
Memory-bound elementwise map over 64x510x510 elements.

Strategy (hardcoded, self-contained):
  - Pure data parallel over batch: 8 images per core x 8 cores.
  - Host slices the needed top-left 510x510 region, flattens each core's
    8 images to one [128, 16384] f32 array (zero-padded tail) so the
    device sees perfectly regular, full-partition DMAs.
  - Device per tile: ACT Sin (cos via phase shift, arg reduced into
    [-pi, pi] on host), ACT Tanh (sigmoid(v) = 0.5*tanh(v/2) + 0.5 --
    Sin and Tanh share one activation-table set, Sigmoid does not),
    DVE tensor_scalar for the final 0.5*h + 0.5 affine.
  - All runtime scalars (phase, sign, threshold) are fed via a tiny
    "consts" input tensor so the compiled program is value-independent.
"""

import math

import numpy as np

B, H, W = 64, 512, 512
KS = 3
OH = OW = H - KS + 1          # 510
NCORES = 8
BPC = B // NCORES             # images per core
P = 128                       # SBUF partitions
ELEMS = BPC * OH * OW         # 2,080,800 elements per core
FREE = 16384                  # padded free dim; P*FREE = 2,097,152 >= ELEMS
TILE = 2048                   # free-dim elements per DMA tile (1 MiB tiles)
NT = FREE // TILE

PROFILE = False               # set True to capture an NTFF profile
LAST_RESULTS = None           # BassKernelResults of the last run

_prog_cache = {}


def _get_program():
    if "nc" in _prog_cache:
        return _prog_cache["nc"]

    import concourse.bass as bass
    import concourse.tile as tile
    from concourse import bacc, mybir

    f32 = mybir.dt.float32
    nc = bacc.Bacc(
        "TRN2", target_bir_lowering=False, debug=False, num_devices=NCORES
    )
    x_d = nc.dram_tensor("x", [P, FREE], f32, kind="ExternalInput")
    c_d = nc.dram_tensor("consts", [P, 4], f32, kind="ExternalInput")
    o_d = nc.dram_tensor("out", [P, FREE], f32, kind="ExternalOutput")

    with tile.TileContext(nc) as tc:
        with (
            tc.tile_pool(name="cst", bufs=1) as cst_pool,
            tc.tile_pool(name="xin", bufs=4) as xin_pool,
            tc.tile_pool(name="mid", bufs=3) as mid_pool,
            tc.tile_pool(name="res", bufs=4) as res_pool,
        ):
            cst = cst_pool.tile([P, 4], f32)
            nc.sync.dma_start(cst[:], c_d.ap())
            for i in range(NT):
                xt = xin_pool.tile([P, TILE], f32)
                nc.sync.dma_start(xt[:], x_d.ap()[:, bass.ts(i, TILE)])
                # s = sin(pi*x + cp) = sign * cos(pi*x + w0)
                st = mid_pool.tile([P, TILE], f32)
                nc.scalar.activation(
                    st[:],
                    xt[:],
                    mybir.ActivationFunctionType.Sin,
                    bias=cst[:, 0:1],
                    scale=float(np.pi),
                )
                # h = tanh(0.5*sign*s - 0.5*th)
                ht = mid_pool.tile([P, TILE], f32)
                nc.scalar.activation(
                    ht[:],
                    st[:],
                    mybir.ActivationFunctionType.Tanh,
                    scale=cst[:, 1:2],
                    bias=cst[:, 2:3],
                )
                # out = 0.5*h + 0.5 = sigmoid(sign*s - th)
                ot = res_pool.tile([P, TILE], f32)
                nc.vector.tensor_scalar(
                    ot[:],
                    ht[:],
                    0.5,
                    0.5,
                    mybir.AluOpType.mult,
                    mybir.AluOpType.add,
                )
                nc.sync.dma_start(o_d.ap()[:, bass.ts(i, TILE)], ot[:])
    nc.compile()
    _prog_cache["nc"] = nc
    return nc


def build_in_maps(x, weight, threshold):
    """Host-side shard + pack: full inputs -> per-core input maps."""
    x = np.asarray(x, dtype=np.float32)
    w0 = float(np.asarray(weight).reshape(-1)[0])
    th = float(np.asarray(threshold).reshape(-1)[0])

    # cos(pi*x + w0) = sign * sin(pi*x + cp) with pi*x + cp in [-pi, pi]
    # (x in [0, 1)). ScalarE Sin is only valid on [-pi, pi].
    c = w0 + math.pi / 2.0
    k = round(c / (2.0 * math.pi))
    cp = c - 2.0 * math.pi * k          # in [-pi, pi]
    sign = 1.0
    if cp > 0.0:
        sign, cp = -1.0, cp - math.pi   # now cp in (-pi, 0]

    consts = np.zeros((P, 4), np.float32)
    consts[:, 0] = cp
    consts[:, 1] = 0.5 * sign
    consts[:, 2] = -0.5 * th

    # [64,1,512,512] -> per-core [128, FREE]
    xs = x[:, 0, :OH, :OW].reshape(NCORES, ELEMS)
    xpad = np.zeros((NCORES, P * FREE), np.float32)
    xpad[:, :ELEMS] = xs
    xpad = xpad.reshape(NCORES, P, FREE)
    return [{"x": xpad[i], "consts": consts} for i in range(NCORES)]


def assemble_output(results):
    """Per-core result dicts -> full [64,1,510,510] output."""
    out = np.empty((B, OH, OW), np.float32)
    for i in range(NCORES):
        out[i * BPC : (i + 1) * BPC] = (
            results[i]["out"].reshape(-1)[:ELEMS].reshape(BPC, OH, OW)
        )
    return out[:, None, :, :]


def kernel(x, weight, threshold):
    global LAST_RESULTS
    from concourse.bass_utils import run_bass_kernel_spmd

    in_maps = build_in_maps(x, weight, threshold)
    nc = _get_program()
    LAST_RESULTS = run_bass_kernel_spmd(
        nc, in_maps, list(range(NCORES)), trace=PROFILE
    )
    return assemble_output(LAST_RESULTS.results)


# revision 6
# speedup vs baseline: 29.0016x; 29.0016x over previous
"""Trainium2 Bass kernel for:
    out = sigmoid(cos(pi * x[:, 0, :510, :510] + weight[0]) - threshold[0])[:, None]

x: [64, 1, 512, 512] f32, weight: [9] f32, threshold: [1] f32.
Memory-bound elementwise map over 64x510x510 elements.

Strategy (hardcoded, self-contained):
  - Pure data parallel over batch: 8 images per core x 8 cores.
  - Host slices the needed top-left 510x510 region, flattens each core's
    8 images to one [128, 16384] f32 array (zero-padded tail) so the
    device sees perfectly regular, full-partition DMAs.
  - Device per tile: ACT Sin (cos via phase shift, arg reduced into
    [-pi, pi] on host), ACT Tanh (sigmoid(v) = 0.5*tanh(v/2) + 0.5 --
    Sin and Tanh share one activation-table set, Sigmoid does not),
    DVE tensor_scalar for the final 0.5*h + 0.5 affine.
  - All runtime scalars (phase, sign, threshold) are fed via a tiny
    "consts" input tensor so the compiled program is value-independent.
"""

import math

import numpy as np

B, H, W = 64, 512, 512
KS = 3
OH = OW = H - KS + 1          # 510
NCORES = 8
BPC = B // NCORES             # images per core
P = 128                       # SBUF partitions
ELEMS = BPC * OH * OW         # 2,080,800 elements per core
FREE = 16384                  # padded free dim; P*FREE = 2,097,152 >= ELEMS
TILE = 2048                   # free-dim elements per DMA tile (1 MiB tiles)
NT = FREE // TILE

PROFILE = False               # set True to capture an NTFF profile
LAST_RESULTS = None           # BassKernelResults of the last run

_prog_cache = {}


def _get_program(repeat=1):
    if repeat in _prog_cache:
        return _prog_cache[repeat]

    import concourse.bass as bass
    import concourse.tile as tile
    from concourse import bacc, mybir

    f32 = mybir.dt.float32
    nc = bacc.Bacc(
        "TRN2", target_bir_lowering=False, debug=False, num_devices=NCORES
    )
    x_d = nc.dram_tensor("x", [P, FREE], f32, kind="ExternalInput")
    c_d = nc.dram_tensor("consts", [P, 4], f32, kind="ExternalInput")
    o_d = nc.dram_tensor("out", [P, FREE], f32, kind="ExternalOutput")

    with tile.TileContext(nc) as tc:
        with (
            tc.tile_pool(name="cst", bufs=1) as cst_pool,
            tc.tile_pool(name="xin", bufs=4) as xin_pool,
            tc.tile_pool(name="mid", bufs=3) as mid_pool,
            tc.tile_pool(name="res", bufs=4) as res_pool,
        ):
            cst = cst_pool.tile([P, 4], f32)
            nc.sync.dma_start(cst[:], c_d.ap())

            def body():
                for i in range(NT):
                    xt = xin_pool.tile([P, TILE], f32)
                    nc.sync.dma_start(xt[:], x_d.ap()[:, bass.ts(i, TILE)])
                    # s = sin(pi*x + cp) = sign * cos(pi*x + w0)
                    st = mid_pool.tile([P, TILE], f32)
                    nc.scalar.activation(
                        st[:],
                        xt[:],
                        mybir.ActivationFunctionType.Sin,
                        bias=cst[:, 0:1],
                        scale=float(np.pi),
                    )
                    # h = tanh(0.5*sign*s - 0.5*th)
                    ht = mid_pool.tile([P, TILE], f32)
                    nc.scalar.activation(
                        ht[:],
                        st[:],
                        mybir.ActivationFunctionType.Tanh,
                        scale=cst[:, 1:2],
                        bias=cst[:, 2:3],
                    )
                    # out = 0.5*h + 0.5 = sigmoid(sign*s - th)
                    ot = res_pool.tile([P, TILE], f32)
                    nc.vector.tensor_scalar(
                        ot[:],
                        ht[:],
                        0.5,
                        0.5,
                        mybir.AluOpType.mult,
                        mybir.AluOpType.add,
                    )
                    nc.sync.dma_start(o_d.ap()[:, bass.ts(i, TILE)], ot[:])

            if repeat == 1:
                body()
            else:
                # benchmark-only: loop the identical workload on-device so
                # per-dispatch overhead amortizes out of the measurement
                with tc.For_i(0, repeat, 1):
                    body()
    nc.compile()
    _prog_cache[repeat] = nc
    return nc


def build_in_maps(x, weight, threshold):
    """Host-side shard + pack: full inputs -> per-core input maps."""
    x = np.asarray(x, dtype=np.float32)
    w0 = float(np.asarray(weight).reshape(-1)[0])
    th = float(np.asarray(threshold).reshape(-1)[0])

    # cos(pi*x + w0) = sign * sin(pi*x + cp) with pi*x + cp in [-pi, pi]
    # (x in [0, 1)). ScalarE Sin is only valid on [-pi, pi].
    c = w0 + math.pi / 2.0
    k = round(c / (2.0 * math.pi))
    cp = c - 2.0 * math.pi * k          # in [-pi, pi]
    sign = 1.0
    if cp > 0.0:
        sign, cp = -1.0, cp - math.pi   # now cp in (-pi, 0]

    consts = np.zeros((P, 4), np.float32)
    consts[:, 0] = cp
    consts[:, 1] = 0.5 * sign
    consts[:, 2] = -0.5 * th

    # [64,1,512,512] -> per-core [128, FREE]
    xs = x[:, 0, :OH, :OW].reshape(NCORES, ELEMS)
    xpad = np.zeros((NCORES, P * FREE), np.float32)
    xpad[:, :ELEMS] = xs
    xpad = xpad.reshape(NCORES, P, FREE)
    return [{"x": xpad[i], "consts": consts} for i in range(NCORES)]


def assemble_output(results):
    """Per-core result dicts -> full [64,1,510,510] output."""
    out = np.empty((B, OH, OW), np.float32)
    for i in range(NCORES):
        out[i * BPC : (i + 1) * BPC] = (
            results[i]["out"].reshape(-1)[:ELEMS].reshape(BPC, OH, OW)
        )
    return out[:, None, :, :]


def kernel(x, weight, threshold):
    global LAST_RESULTS
    from concourse.bass_utils import run_bass_kernel_spmd

    in_maps = build_in_maps(x, weight, threshold)
    nc = _get_program()
    LAST_RESULTS = run_bass_kernel_spmd(
        nc, in_maps, list(range(NCORES)), trace=PROFILE
    )
    return assemble_output(LAST_RESULTS.results)


# revision 33
# speedup vs baseline: 32.3287x; 1.1147x over previous
"""Trainium2 Bass kernel for:
    out = sigmoid(cos(pi * x[:, 0, :510, :510] + weight[0]) - threshold[0])[:, None]

x: [64, 1, 512, 512] f32, weight: [9] f32, threshold: [1] f32.
Memory-bound elementwise map over 64x510x510 elements.

Strategy (hardcoded, self-contained):
  - Pure data parallel over batch: 8 images per core x 8 cores.
  - Host slices the needed top-left 510x510 region, flattens each core's
    8 images to one [128, 16384] f32 array (zero-padded tail) so the
    device sees perfectly regular, full-partition DMAs.
  - Device per tile: ACT Sin (cos via phase shift, arg reduced into
    [-pi, pi] on host), ACT Tanh (sigmoid(v) = 0.5*tanh(v/2) + 0.5 --
    Sin and Tanh share one activation-table set, Sigmoid does not),
    DVE tensor_scalar for the final 0.5*h + 0.5 affine.
  - All runtime scalars (phase, sign, threshold) are fed via a tiny
    "consts" input tensor so the compiled program is value-independent.
"""

import math

import numpy as np

B, H, W = 64, 512, 512
KS = 3
OH = OW = H - KS + 1          # 510
NCORES = 8
BPC = B // NCORES             # images per core
P = 128                       # SBUF partitions
ELEMS = BPC * OH * OW         # 2,080,800 elements per core
FREE = 16384                  # padded free dim; P*FREE = 2,097,152 >= ELEMS
TILE = 2048                   # free-dim elements per DMA tile (1 MiB tiles)
NT = FREE // TILE

PROFILE = False               # set True to capture an NTFF profile
LAST_RESULTS = None           # BassKernelResults of the last run

_prog_cache = {}


def _get_program(
    repeat=1,
    tile_free=TILE,
    xin_bufs=4,
    mid_bufs=3,
    res_bufs=4,
    store_eng="sync",
    staggered=False,
    inplace=False,
    mode="full",
    free=FREE,
    ramp=None,
    load_sz=None,
    comp_sz=None,
    store_sz=None,
    cst_eng="sync",
    affine="vector",
    emit="interleaved",
    layout="rowmajor",
    ndev=NCORES,
):
    key = (
        repeat, tile_free, xin_bufs, mid_bufs, res_bufs, store_eng, staggered,
        inplace, mode, free, ramp, load_sz, comp_sz, store_sz, cst_eng,
        affine, emit, layout, ndev,
    )
    if key in _prog_cache:
        return _prog_cache[key]

    import concourse.bass as bass
    import concourse.tile as tile
    from concourse import bacc, mybir

    if ramp is not None:
        assert sum(ramp) == free
        sizes = list(ramp)
    else:
        assert free % tile_free == 0
        sizes = [tile_free] * (free // tile_free)
    offs = [sum(sizes[:i]) for i in range(len(sizes))]
    slot = max(sizes)

    f32 = mybir.dt.float32
    nc = bacc.Bacc(
        "TRN2", target_bir_lowering=False, debug=False, num_devices=ndev
    )
    if layout == "tilemajor":
        # each [P, tile_free] tile occupies one fully-contiguous DRAM block
        assert ramp is None and mode == "full" and load_sz is None
        nt_ = free // tile_free
        x_d = nc.dram_tensor("x", [nt_ * P, tile_free], f32, kind="ExternalInput")
        o_d = nc.dram_tensor("out", [nt_ * P, tile_free], f32, kind="ExternalOutput")
    else:
        x_d = nc.dram_tensor("x", [P, free], f32, kind="ExternalInput")
        o_d = nc.dram_tensor("out", [P, free], f32, kind="ExternalOutput")
    c_d = nc.dram_tensor("consts", [P, 4], f32, kind="ExternalInput")

    with tile.TileContext(nc) as tc:
        with (
            tc.tile_pool(name="cst", bufs=1) as cst_pool,
            tc.tile_pool(name="xin", bufs=xin_bufs) as xin_pool,
            tc.tile_pool(name="mid", bufs=mid_bufs) as mid_pool,
            tc.tile_pool(name="res", bufs=res_bufs) as res_pool,
        ):
            cst = cst_pool.tile([P, 4], f32)
            getattr(nc, cst_eng).dma_start(cst[:], c_d.ap())
            store = getattr(nc, store_eng)

            def do_affine(dst, src):
                # out = 0.5*h + 0.5 = sigmoid(sign*s - th)
                if affine == "scalar":
                    nc.scalar.activation(
                        dst,
                        src,
                        mybir.ActivationFunctionType.Identity,
                        bias=cst[:, 3:4],
                        scale=0.5,
                    )
                else:
                    getattr(nc, affine).tensor_scalar(
                        dst,
                        src,
                        0.5,
                        0.5,
                        mybir.AluOpType.mult,
                        mybir.AluOpType.add,
                    )

            def body():
                if mode == "storeonly":
                    zt = xin_pool.tile([P, slot], f32)
                    nc.vector.memset(zt[:], 0.5)
                    for off, sz in zip(offs, sizes):
                        store.dma_start(
                            o_d.ap()[:, off : off + sz], zt[:, 0:sz]
                        )
                    return
                def src_ap(idx, off, sz):
                    if layout == "tilemajor":
                        return x_d.ap()[idx * P : (idx + 1) * P, :]
                    return x_d.ap()[:, off : off + sz]

                def dst_ap(idx, off, sz):
                    if layout == "tilemajor":
                        return o_d.ap()[idx * P : (idx + 1) * P, :]
                    return o_d.ap()[:, off : off + sz]

                pre = []
                if emit == "phases":
                    for idx, (off, sz) in enumerate(zip(offs, sizes)):
                        xt = xin_pool.tile([P, slot], f32)
                        nc.sync.dma_start(xt[:, 0:sz], src_ap(idx, off, sz))
                        pre.append(xt)
                for idx, (off, sz) in enumerate(zip(offs, sizes)):
                    if emit == "phases":
                        xt = pre[idx]
                    else:
                        xt = xin_pool.tile([P, slot], f32)
                        nc.sync.dma_start(xt[:, 0:sz], src_ap(idx, off, sz))
                    if mode == "loadonly":
                        continue
                    # s = sin(pi*x + cp) = sign * cos(pi*x + w0)
                    st = xt if inplace else mid_pool.tile([P, slot], f32)
                    nc.scalar.activation(
                        st[:, 0:sz],
                        xt[:, 0:sz],
                        mybir.ActivationFunctionType.Sin,
                        bias=cst[:, 0:1],
                        scale=float(np.pi),
                    )
                    # h = tanh(0.5*sign*s - 0.5*th)
                    ht = st if inplace else mid_pool.tile([P, slot], f32)
                    nc.scalar.activation(
                        ht[:, 0:sz],
                        st[:, 0:sz],
                        mybir.ActivationFunctionType.Tanh,
                        scale=cst[:, 1:2],
                        bias=cst[:, 2:3],
                    )
                    ot = ht if inplace else res_pool.tile([P, slot], f32)
                    do_affine(ot[:, 0:sz], ht[:, 0:sz])
                    store.dma_start(dst_ap(idx, off, sz), ot[:, 0:sz])

            def body_split():
                # big loads for DMA efficiency, fine-grained in-place
                # compute on sub-slices, mid-size stores
                assert free % load_sz == 0
                assert load_sz % comp_sz == 0 and load_sz % store_sz == 0
                assert store_sz % comp_sz == 0
                for ib in range(free // load_sz):
                    base = ib * load_sz
                    xt = xin_pool.tile([P, load_sz], f32)
                    nc.sync.dma_start(
                        xt[:], x_d.ap()[:, base : base + load_sz]
                    )
                    for j in range(load_sz // comp_sz):
                        sl = xt[:, j * comp_sz : (j + 1) * comp_sz]
                        nc.scalar.activation(
                            sl,
                            sl,
                            mybir.ActivationFunctionType.Sin,
                            bias=cst[:, 0:1],
                            scale=float(np.pi),
                        )
                        nc.scalar.activation(
                            sl,
                            sl,
                            mybir.ActivationFunctionType.Tanh,
                            scale=cst[:, 1:2],
                            bias=cst[:, 2:3],
                        )
                        nc.vector.tensor_scalar(
                            sl,
                            sl,
                            0.5,
                            0.5,
                            mybir.AluOpType.mult,
                            mybir.AluOpType.add,
                        )
                        if ((j + 1) * comp_sz) % store_sz == 0:
                            so = ((j + 1) * comp_sz // store_sz - 1) * store_sz
                            store.dma_start(
                                o_d.ap()[:, base + so : base + so + store_sz],
                                xt[:, so : so + store_sz],
                            )

            run = body_split if load_sz is not None else body
            if repeat == 1:
                run()
            else:
                # benchmark-only: loop the identical workload on-device so
                # per-dispatch overhead amortizes out of the measurement
                with tc.For_i(0, repeat, 1, staggered_reset=staggered):
                    run()
    nc.compile()
    _prog_cache[key] = nc
    return nc


def build_in_maps(x, weight, threshold, free=FREE, layout="rowmajor",
                  tile_free=TILE):
    """Host-side shard + pack: full inputs -> per-core input maps."""
    x = np.asarray(x, dtype=np.float32)
    w0 = float(np.asarray(weight).reshape(-1)[0])
    th = float(np.asarray(threshold).reshape(-1)[0])

    # cos(pi*x + w0) = sign * sin(pi*x + cp) with pi*x + cp in [-pi, pi]
    # (x in [0, 1)). ScalarE Sin is only valid on [-pi, pi].
    c = w0 + math.pi / 2.0
    k = round(c / (2.0 * math.pi))
    cp = c - 2.0 * math.pi * k          # in [-pi, pi]
    sign = 1.0
    if cp > 0.0:
        sign, cp = -1.0, cp - math.pi   # now cp in (-pi, 0]

    consts = np.zeros((P, 4), np.float32)
    consts[:, 0] = cp
    consts[:, 1] = 0.5 * sign
    consts[:, 2] = -0.5 * th
    consts[:, 3] = 0.5

    # [64,1,512,512] -> per-core [128, free]
    xs = x[:, 0, :OH, :OW].reshape(NCORES, ELEMS)
    xpad = np.zeros((NCORES, P * free), np.float32)
    xpad[:, :ELEMS] = xs
    xpad = xpad.reshape(NCORES, P, free)
    if layout == "tilemajor":
        nt = free // tile_free
        xpad = np.ascontiguousarray(
            xpad.reshape(NCORES, P, nt, tile_free).transpose(0, 2, 1, 3)
        ).reshape(NCORES, nt * P, tile_free)
    return [{"x": xpad[i], "consts": consts} for i in range(NCORES)]


def assemble_output(results):
    """Per-core result dicts -> full [64,1,510,510] output.

    Detects the DRAM layout from the result shape: [P, free] is
    row-major; [nt*P, tile_free] is tile-major.
    """
    out = np.empty((B, OH, OW), np.float32)
    for i in range(NCORES):
        r = results[i]["out"]
        if r.shape[0] != P:
            nt = r.shape[0] // P
            r = r.reshape(nt, P, r.shape[1]).transpose(1, 0, 2)
        out[i * BPC : (i + 1) * BPC] = (
            r.reshape(-1)[:ELEMS].reshape(BPC, OH, OW)
        )
    return out[:, None, :, :]


# Tuned on hardware (A/B slope benchmarks, see bench_ab.py):
#   - 4 tiles/direction of [128, 4096] f32 (2 MiB DMAs) beat 1/2/5/6/8-tile
#     layouts; in-place compute (one SBUF pool, 8 bufs) beat split pools.
#   - DVE affine beat ACT-Identity (ACT saturates) and GpSimd.
#   - measured ~52-56 us/exec steady state ~= load-only + store-only DMA
#     time (~310-320 GB/s/core of the ~358 GB/s HBM limit); compute fully
#     hidden. Tile-major DRAM layout, ramped tiles, split load/compute
#     granularity, and HWDGE ring splits were all neutral or worse.
BEST_CFG = dict(
    tile_free=4096, inplace=True, xin_bufs=8, mid_bufs=1, res_bufs=1, ndev=1
)


def kernel(x, weight, threshold):
    global LAST_RESULTS
    from concourse.bass_utils import run_bass_kernel_spmd

    in_maps = build_in_maps(x, weight, threshold)
    nc = _get_program(**BEST_CFG)
    LAST_RESULTS = run_bass_kernel_spmd(
        nc, in_maps, list(range(NCORES)), trace=PROFILE
    )
    return assemble_output(LAST_RESULTS.results)
